# revision 1
# baseline (speedup 1.0000x reference)
"""Trainium2 Bass kernel for nn_AttentionNestedTensor (ragged packed attention).

Sharding: head-parallel across 8 cores (16 heads -> 2 heads/core).
Each core:
  - projects q/k/v for ALL tokens but only its 2 heads (slice of wq/wk/wv)
  - fused qk rmsnorm (over head_dim=64)
  - block-diagonal ragged attention for its 2 heads (exp without max-subtract:
    scores are bounded by ||qn||*||kn||/sqrt(hd) = hd/sqrt(hd) * max|gq*gk| ~ 8,
    so fp32 exp is safe)
  - two half-chunk AllToAlls exchange attention outputs (token-chunk <->
    head-group), the second overlapping the first half's output projection;
    each core ends with ALL heads for its 1024-token chunk
  - local output projection (full wo) + bias -> [1024, 1024] chunk
Host concatenates the 8 chunks.

Compute dtype bf16 (fp32 PSUM accumulation).  Layouts / tricks:
  - host passes query.T / key_value.T (pre-cast bf16) so the contraction dim
    lands on SBUF partitions with no device transposes and half the DMA bytes
  - q/k are projected weight-stationary into [head_dim, tokens] (what the
    score matmuls want); v is projected the same way then PE-transposed to
    [tokens, head_dim] (what the PV matmul wants)
  - rmsnorm stats: sum(q^2) via a block-diagonal ones matmul (cross-partition
    reduce on PE), rsqrt broadcast back across partitions via a tiny K=2
    matmul that also folds in the g scale; the squaring runs on GpSimd so
    ScalarE keeps its Exp table hot
  - scores are built TRANSPOSED ([kv, q]) so softmax needs no transposes:
    exp runs without max-subtraction (rmsnorm bounds |score| <= sqrt(hd)),
    the denominator comes free as an extra ones-column in the PV lhsT, and
    the final 1/l is a partition_broadcast (GpSimd) + one DVE multiply
  - attention is emitted per segment as soon as its tiles are projected, so
    exp work (the ScalarE floor, ~135us/core) spreads across the DMA-bound
    projection phase
  - per-engine steady state (cost-model sim): PE ~63%, ACT ~49%, DVE ~36%,
    total ~429us/core predicted (collectives conservatively modeled; the
    real intra-chip A2A is ~3x cheaper than the model's 15us+bytes/40GBps).
"""

import os
import sys

import numpy as np

try:
    import concourse.bass as bass  # noqa: F401
except ImportError:
    sys.path.insert(0, "/opt/trn_rl_repo")

import ml_dtypes

BF16 = ml_dtypes.bfloat16

EMBED = 1024
HEADS = 16
HD = EMBED // HEADS  # 64
EPS = 1e-6
NCORES = 8
HPC = HEADS // NCORES  # heads per core = 2
DPC = HPC * HD  # dims per core = 128
KT = EMBED // 128  # contraction tiles = 8

# "a2a": AllToAll on device + local output projection (device does everything
#        but the final concat).  "hostsum": each core returns a full-length
#        partial output projection; host sums.  a2a is the default.
MODE = os.environ.get("ATTN_KERNEL_MODE", "a2a")

_BUILD_CACHE: dict = {}
LAST_RESULT = None  # BassKernelResults of the most recent run (for test.py)


def _ichunks(n, step):
    out = []
    i = 0
    while i < n:
        out.append((i, min(step, n - i)))
        i += step
    return out


def _build(lq, lkv, mode):
    import concourse.bass as bass
    import concourse.mybir as mybir
    import concourse.tile as tile
    from concourse import bacc

    dt = mybir.dt
    f32 = dt.float32
    bf16 = dt.bfloat16
    AF = mybir.ActivationFunctionType

    T = int(sum(lq))
    TKV = int(sum(lkv))
    qoff = np.concatenate([[0], np.cumsum(lq)]).astype(int)
    koff = np.concatenate([[0], np.cumsum(lkv)]).astype(int)
    nseg = len(lq)
    assert T % 512 == 0 and TKV % 128 == 0
    for x in list(lq) + list(lkv):
        assert x % 128 == 0, "segment lengths must be multiples of 128"
    NT = T // 512  # projection tiles (512 tokens each)
    NKV128 = TKV // 128
    CHUNK = T // NCORES  # tokens per core after a2a

    nc = bacc.Bacc("TRN2", target_bir_lowering=False, debug=False)

    # ---- kernel I/O ----
    xqT_d = nc.declare_dram_parameter("xqT", [EMBED, T], bf16, isOutput=False)
    xkvT_d = nc.declare_dram_parameter("xkvT", [EMBED, TKV], bf16, isOutput=False)
    wq_d = nc.declare_dram_parameter("wqT", [EMBED, DPC], bf16, isOutput=False)
    wk_d = nc.declare_dram_parameter("wkT", [EMBED, DPC], bf16, isOutput=False)
    wv_d = nc.declare_dram_parameter("wvT", [EMBED, DPC], bf16, isOutput=False)
    e2ones_d = nc.declare_dram_parameter("e2ones", [128, HPC], bf16, isOutput=False)
    e2gq_d = nc.declare_dram_parameter("e2gq", [HPC, 128], bf16, isOutput=False)
    e2gk_d = nc.declare_dram_parameter("e2gk", [HPC, 128], bf16, isOutput=False)
    if mode == "a2a":
        wo_d = nc.declare_dram_parameter("woT", [EMBED, EMBED], bf16, isOutput=False)
        bo_d = nc.declare_dram_parameter("bo", [EMBED], f32, isOutput=False)
        out_d = nc.declare_dram_parameter("out", [CHUNK, EMBED], f32, isOutput=True)
    else:
        wo_d = nc.declare_dram_parameter("woTc", [DPC, EMBED], bf16, isOutput=False)
        out_d = nc.declare_dram_parameter("out", [T, EMBED], f32, isOutput=True)

    with tile.TileContext(nc) as tc:
        from contextlib import ExitStack

        ctx = ExitStack()
        with ctx:
            singles = ctx.enter_context(tc.tile_pool(name="singles", bufs=1))
            persist = ctx.enter_context(tc.tile_pool(name="persist", bufs=1))
            io = ctx.enter_context(tc.tile_pool(name="io", bufs=2))
            tmp = ctx.enter_context(tc.tile_pool(name="tmp", bufs=4))
            small = ctx.enter_context(tc.tile_pool(name="small", bufs=4))
            epool = ctx.enter_context(tc.tile_pool(name="epool", bufs=6))
            outst = ctx.enter_context(tc.tile_pool(name="outst", bufs=2))
            pacc = ctx.enter_context(tc.tile_pool(name="pacc", bufs=2, space="PSUM"))
            pstat = pacc
            pst = ctx.enter_context(tc.tile_pool(name="pst", bufs=2, space="PSUM"))
            ppv = ctx.enter_context(tc.tile_pool(name="ppv", bufs=2, space="PSUM"))
            dram = ctx.enter_context(tc.tile_pool(name="dram", bufs=1, space="DRAM"))

            # ---- load constants ----
            wq_s = singles.tile([128, KT, DPC], bf16, tag="wq")
            wk_s = singles.tile([128, KT, DPC], bf16, tag="wk")
            wv_s = singles.tile([128, KT, DPC], bf16, tag="wv")
            for w_s, w_d in ((wq_s, wq_d), (wk_s, wk_d), (wv_s, wv_d)):
                nc.sync.dma_start(
                    out=w_s,
                    in_=w_d[:, :].rearrange("(k p) m -> p k m", p=128),
                )
            e2ones_s = singles.tile([128, HPC], bf16, tag="e2ones")
            nc.sync.dma_start(out=e2ones_s, in_=e2ones_d[:, :])
            e2gq_s = singles.tile([HPC, 128], bf16, tag="e2gq")
            nc.sync.dma_start(out=e2gq_s, in_=e2gq_d[:, :])
            e2gk_s = singles.tile([HPC, 128], bf16, tag="e2gk")
            nc.sync.dma_start(out=e2gk_s, in_=e2gk_d[:, :])
            eps_s = singles.tile([HPC, 1], f32, tag="eps")
            nc.vector.memset(eps_s, EPS)
            identity = singles.tile([128, 128], bf16, tag="identity")
            from concourse.masks import make_identity

            make_identity(nc, identity)

            if mode == "a2a":
                wo_s = singles.tile([128, KT, EMBED], bf16, tag="wo")
                bo_s = singles.tile([128, EMBED], f32, tag="bo")

                def load_wo():
                    nc.sync.dma_start(
                        out=wo_s,
                        in_=wo_d[:, :].rearrange("(k p) m -> p k m", p=128),
                    )
                    bo_ap = bo_d[:]
                    bo_bcast = bass.AP(
                        tensor=bo_ap.tensor,
                        offset=bo_ap.offset,
                        ap=[[0, 128]] + list(bo_ap.ap),
                    )
                    nc.sync.dma_start(out=bo_s, in_=bo_bcast)
            else:
                wo_s = singles.tile([128, EMBED], bf16, tag="wo")

                def load_wo():
                    nc.sync.dma_start(out=wo_s, in_=wo_d[:, :])

            # ---- persistent activations ----
            qnT = persist.tile([128, T], bf16, tag="qnT")  # [2*64 qdim, T]
            knT = persist.tile([128, TKV], bf16, tag="knT")
            # v with a ones column per head: [tok_part, tok_tile, 65*HPC]
            v_s = persist.tile([128, NKV128, 65 * HPC], bf16, tag="v")
            nc.vector.memset(v_s[:, :, 64:65], 1.0)
            nc.vector.memset(v_s[:, :, 129:130], 1.0)
            attnT = persist.tile([128, T], bf16, tag="attnT")

            # ---- projections + norm, per 512-token tile ----
            def norm_and_store(dst, acc, gcol, t0, tlen):
                # acc: PSUM [128, tlen] projection result (transposed layout).
                # Copy to SBUF immediately so the PSUM slot frees fast and the
                # next projection's matmuls keep PE dense.
                qt = tmp.tile([128, 512], bf16, tag="qt")
                nc.vector.tensor_copy(out=qt[:, :tlen], in_=acc)
                sq = tmp.tile([128, 512], bf16, tag="sq")
                nc.gpsimd.tensor_mul(
                    out=sq[:, :tlen], in0=qt[:, :tlen], in1=qt[:, :tlen]
                )
                pm = pst.tile([HPC, 512], f32, tag="st", name="pm")
                nc.tensor.matmul(
                    out=pm[:, :tlen], lhsT=e2ones_s, rhs=sq[:, :tlen],
                    start=True, stop=True,
                )
                sm = small.tile([HPC, 512], f32, tag="sm")
                nc.scalar.activation(
                    out=sm[:, :tlen], in_=pm[:, :tlen], func=AF.Sqrt,
                    bias=eps_s[:, :], scale=1.0 / HD,
                )
                rq = small.tile([HPC, 512], f32, tag="rq")
                nc.vector.reciprocal(out=rq[:, :tlen], in_=sm[:, :tlen])
                rqb = small.tile([HPC, 512], bf16, tag="rqb")
                nc.vector.tensor_copy(out=rqb[:, :tlen], in_=rq[:, :tlen])
                pb = pst.tile([128, 512], f32, tag="st", name="pb")
                nc.tensor.matmul(
                    out=pb[:, :tlen], lhsT=gcol, rhs=rqb[:, :tlen],
                    start=True, stop=True,
                )
                nc.vector.tensor_mul(
                    out=dst[:, t0:t0 + tlen], in0=qt[:, :tlen], in1=pb[:, :tlen]
                )

            # ---- ragged block-diagonal attention (emitted per segment as
            # soon as its projections are emitted, so exp work spreads) ----
            def emit_attention(s):
                Lq, Lkv = int(lq[s]), int(lkv[s])
                q0, k0 = int(qoff[s]), int(koff[s])
                for i0, ilen in _ichunks(Lq, 512):
                    pvh = [
                        ppv.tile([65, 512], f32, tag="pv", name=f"pv{h}")
                        for h in range(HPC)
                    ]
                    njt = Lkv // 128
                    for jt in range(njt):
                        j0 = k0 + jt * 128
                        pS = pst.tile([128, 2 * 512], f32, tag="st")
                        for h in range(HPC):
                            nc.tensor.matmul(
                                out=pS[:, 512 * h:512 * h + ilen],
                                lhsT=knT[64 * h:64 * (h + 1), j0:j0 + 128],
                                rhs=qnT[64 * h:64 * (h + 1), q0 + i0:q0 + i0 + ilen],
                                start=True, stop=True,
                            )
                        E = epool.tile([128, 2 * 512], bf16, tag="E")
                        if ilen == 512:
                            nc.scalar.activation(
                                out=E, in_=pS, func=AF.Exp,
                                scale=1.0 / float(np.sqrt(HD)),
                            )
                        else:
                            for h in range(HPC):
                                nc.scalar.activation(
                                    out=E[:, 512 * h:512 * h + ilen],
                                    in_=pS[:, 512 * h:512 * h + ilen],
                                    func=AF.Exp,
                                    scale=1.0 / float(np.sqrt(HD)),
                                )
                        for h in range(HPC):
                            nc.tensor.matmul(
                                out=pvh[h][:, :ilen],
                                lhsT=v_s[:, j0 // 128, 65 * h:65 * (h + 1)],
                                rhs=E[:, 512 * h:512 * h + ilen],
                                start=(jt == 0), stop=(jt == njt - 1),
                            )
                    for h in range(HPC):
                        linv = small.tile([1, 512], f32, tag="linv")
                        nc.vector.reciprocal(
                            out=linv[:, :ilen], in_=pvh[h][64:65, :ilen]
                        )
                        lb = tmp.tile([64, 512], f32, tag="lb")
                        nc.gpsimd.partition_broadcast(
                            lb[:, :ilen], linv[:, :ilen], channels=64
                        )
                        nc.vector.tensor_mul(
                            out=attnT[64 * h:64 * (h + 1), q0 + i0:q0 + i0 + ilen],
                            in0=pvh[h][0:64, :ilen],
                            in1=lb[:, :ilen],
                        )


            next_seg = [0]
            pending_attn = []
            next_chunk = [0]
            if mode == "a2a":
                assert CHUNK % 256 == 0
                HC = CHUNK // 2
                a2a_in_a = dram.tile([NCORES, DPC, HC], bf16, tag="a2aina")
                a2a_in_b = dram.tile([NCORES, DPC, HC], bf16, tag="a2ainb")
                a2a_out_a = dram.tile([NCORES, DPC, HC], bf16, tag="a2aouta")
                a2a_out_b = dram.tile([NCORES, DPC, HC], bf16, tag="a2aoutb")

            big_seg = int(np.argmax(np.asarray(lq) * np.asarray(lkv)))

            def stage_a2a_chunks():
                # stage per HALF-chunk so the first collective's inputs are
                # complete before the attention drain finishes
                if mode != "a2a":
                    return
                done_tok = int(qoff[next_seg[0]])
                if next_seg[0] > big_seg:
                    done_tok = min(done_tok, int(qoff[big_seg]))
                while (
                    next_chunk[0] < 2 * NCORES
                    and done_tok >= HC * (next_chunk[0] + 1)
                ):
                    h = next_chunk[0]
                    buf = a2a_in_a if h % 2 == 0 else a2a_in_b
                    nc.sync.dma_start(
                        out=buf[h // 2], in_=attnT[:, HC * h:HC * (h + 1)]
                    )
                    next_chunk[0] += 1
            for t in range(max(NT, TKV // 512)):
                t0 = t * 512
                if t < NT:
                    xq = io.tile([128, KT, 512], bf16, tag="xq")
                    nc.sync.dma_start(
                        out=xq,
                        in_=xqT_d[:, :].rearrange("(k p) t -> p k t", p=128)[
                            :, :, t0:t0 + 512
                        ],
                    )
                    pq = pacc.tile([128, 512], f32, tag="acc")
                    for k in range(KT):
                        nc.tensor.matmul(
                            out=pq, lhsT=wq_s[:, k, :], rhs=xq[:, k, :],
                            start=(k == 0), stop=(k == KT - 1),
                        )
                    norm_and_store(qnT, pq, e2gq_s, t0, 512)
                if t >= TKV // 512:
                    continue
                xkv = io.tile([128, KT, 512], bf16, tag="xkv")
                nc.sync.dma_start(
                    out=xkv,
                    in_=xkvT_d[:, :].rearrange("(k p) t -> p k t", p=128)[
                        :, :, t0:t0 + 512
                    ],
                )
                pk = pacc.tile([128, 512], f32, tag="acc")
                for k in range(KT):
                    nc.tensor.matmul(
                        out=pk, lhsT=wk_s[:, k, :], rhs=xkv[:, k, :],
                        start=(k == 0), stop=(k == KT - 1),
                    )
                norm_and_store(knT, pk, e2gk_s, t0, 512)
                # V: project dim-stationary -> vT [vdim, tok], then PE-transpose
                pvt = pacc.tile([128, 512], f32, tag="acc", name="pvt")
                for k in range(KT):
                    nc.tensor.matmul(
                        out=pvt, lhsT=wv_s[:, k, :], rhs=xkv[:, k, :],
                        start=(k == 0), stop=(k == KT - 1),
                    )
                vts = tmp.tile([128, 512], bf16, tag="vts")
                nc.vector.tensor_copy(out=vts, in_=pvt)
                for s4 in range(4):
                    ptr = ppv.tile([128, 128], bf16, tag="pv", name="ptr")
                    nc.tensor.transpose(
                        ptr, vts[:, 128 * s4:128 * (s4 + 1)], identity
                    )
                    vt = t * 4 + s4
                    nc.vector.tensor_copy(out=v_s[:, vt, 0:64], in_=ptr[:, 0:64])
                    nc.vector.tensor_copy(out=v_s[:, vt, 65:129], in_=ptr[:, 64:128])
                while (
                    next_seg[0] < nseg
                    and qoff[next_seg[0] + 1] <= 512 * (t + 1)
                    and koff[next_seg[0] + 1] <= 512 * (t + 1)
                ):
                    if next_seg[0] != big_seg:
                        pending_attn.append(next_seg[0])
                    next_seg[0] += 1
                for s_ in pending_attn:
                    emit_attention(s_)
                pending_attn.clear()
                stage_a2a_chunks()

            for s_ in range(next_seg[0], nseg):
                if s_ != big_seg:
                    emit_attention(s_)
            emit_attention(big_seg)
            load_wo()

            # ---- output projection ----
            if mode == "a2a":
                for h in range(next_chunk[0], 2 * NCORES):
                    buf = a2a_in_a if h % 2 == 0 else a2a_in_b
                    nc.sync.dma_start(
                        out=buf[h // 2], in_=attnT[:, HC * h:HC * (h + 1)]
                    )
                # two half-size AllToAlls: the second overlaps the first
                # half's output projection
                nc.gpsimd.collective_compute(
                    "AllToAll",
                    mybir.AluOpType.bypass,
                    ins=[a2a_in_a.opt()],
                    outs=[a2a_out_a.opt()],
                    replica_groups=[list(range(NCORES))],
                )
                nc.gpsimd.collective_compute(
                    "AllToAll",
                    mybir.AluOpType.bypass,
                    ins=[a2a_in_b.opt()],
                    outs=[a2a_out_b.opt()],
                    replica_groups=[list(range(NCORES))],
                )
                ao_a = persist.tile([128, NCORES, HC], bf16, tag="aoa")
                ao_b = persist.tile([128, NCORES, HC], bf16, tag="aob")
                for j in range(NCORES):
                    nc.sync.dma_start(out=ao_a[:, j, :], in_=a2a_out_a[j])
                for j in range(NCORES):
                    nc.sync.dma_start(out=ao_b[:, j, :], in_=a2a_out_b[j])
                for ts in range(CHUNK // 128):
                    ao_x = ao_a if ts < HC // 128 else ao_b
                    tsl = ts if ts < HC // 128 else ts - HC // 128
                    os_ = outst.tile([128, EMBED], f32, tag="os")
                    for n2 in range(EMBED // 512):
                        po = pacc.tile([128, 512], f32, tag="acc")
                        for k in range(KT):
                            nc.tensor.matmul(
                                out=po,
                                lhsT=ao_x[:, k, 128 * tsl:128 * (tsl + 1)],
                                rhs=wo_s[:, k, 512 * n2:512 * (n2 + 1)],
                                start=(k == 0), stop=(k == KT - 1),
                            )
                        nc.vector.tensor_add(
                            out=os_[:, 512 * n2:512 * (n2 + 1)], in0=po,
                            in1=bo_s[:, 512 * n2:512 * (n2 + 1)],
                        )
                    nc.sync.dma_start(
                        out=out_d[128 * ts:128 * (ts + 1), :], in_=os_
                    )
            else:
                # partial projection with only this core's 128 attn dims
                for ts in range(T // 128):
                    os_ = outst.tile([128, EMBED], f32, tag="os")
                    for n2 in range(EMBED // 512):
                        po = pacc.tile([128, 512], f32, tag="acc")
                        nc.tensor.matmul(
                            out=po,
                            lhsT=attnT[:, 128 * ts:128 * (ts + 1)],
                            rhs=wo_s[:, 512 * n2:512 * (n2 + 1)],
                            start=True, stop=True,
                        )
                        nc.vector.tensor_copy(
                            out=os_[:, 512 * n2:512 * (n2 + 1)], in_=po
                        )
                    nc.sync.dma_start(
                        out=out_d[128 * ts:128 * (ts + 1), :], in_=os_
                    )

    nc.finalize()
    return nc


_RUNNER_CACHE: dict = {}


def _get_runner(key, nc):
    """Build (once) a cached PJRT executable for `nc` plus metadata.

    Mirrors concourse.bass2jax.run_bass_via_pjrt but keeps the jitted
    callable so repeated runs skip recompilation, and device_puts the
    inputs so repeated runs skip host->device transfer.
    """
    if key in _RUNNER_CACHE:
        return _RUNNER_CACHE[key]
    import jax
    import concourse.mybir as mybir
    from jax.sharding import Mesh, PartitionSpec, NamedSharding
    from jax.experimental.shard_map import shard_map
    from concourse import bass2jax

    bass2jax.install_neuronx_cc_hook()
    partition_name = (
        nc.partition_id_tensor.name if nc.partition_id_tensor else None
    )
    in_names, out_names, out_avals, zero_outs = [], [], [], []
    for alloc in nc.m.functions[0].allocations:
        if not isinstance(alloc, mybir.MemoryLocationSet):
            continue
        name = alloc.memorylocations[0].name
        if alloc.kind == "ExternalInput":
            if name != partition_name:
                in_names.append(name)
        elif alloc.kind == "ExternalOutput":
            shape = tuple(alloc.tensor_shape)
            dtype = mybir.dt.np(alloc.dtype)
            out_names.append(name)
            out_avals.append(jax.core.ShapedArray(shape, dtype))
            zero_outs.append(np.zeros(shape, dtype))
    n_params = len(in_names)
    n_outs = len(out_avals)
    all_in_names = list(in_names) + list(out_names)
    if partition_name is not None:
        all_in_names.append(partition_name)
    donate = tuple(range(n_params, n_params + n_outs))

    def _body(*args):
        operands = list(args)
        if partition_name is not None:
            operands.append(bass2jax.partition_id_tensor())
        outs = bass2jax._bass_exec_p.bind(
            *operands,
            out_avals=tuple(out_avals),
            in_names=tuple(all_in_names),
            out_names=tuple(out_names),
            lowering_input_output_aliases=(),
            sim_require_finite=True,
            sim_require_nnan=True,
            nc=nc,
        )
        return tuple(outs)

    devices = jax.devices()[:NCORES]
    mesh = Mesh(np.asarray(devices), ("core",))
    in_specs = (PartitionSpec("core"),) * (n_params + n_outs)
    out_specs = (PartitionSpec("core"),) * n_outs
    sharded = jax.jit(
        shard_map(
            _body, mesh=mesh, in_specs=in_specs, out_specs=out_specs,
            check_rep=False,
        ),
        donate_argnums=donate,
        keep_unused=True,
    )
    sharding = NamedSharding(mesh, PartitionSpec("core"))

    runner = {
        "sharded": sharded,
        "in_names": in_names,
        "out_names": out_names,
        "out_avals": out_avals,
        "zero_outs": zero_outs,
        "sharding": sharding,
        "n_params": n_params,
    }
    _RUNNER_CACHE[key] = runner
    return runner


def _run(runner, in_maps, n_iters=1):
    """Returns (per-core results list, list of per-iter wall seconds)."""
    import time as _time

    import jax

    concat_in = [
        np.concatenate([np.asarray(m[name]) for m in in_maps], axis=0)
        for name in runner["in_names"]
    ]
    dev_in = [jax.device_put(a, runner["sharding"]) for a in concat_in]
    for a in dev_in:
        a.block_until_ready()
    times = []
    out_arrs = None
    for _ in range(n_iters):
        dev_zeros = [
            jax.device_put(
                np.zeros((NCORES * z.shape[0], *z.shape[1:]), z.dtype),
                runner["sharding"],
            )
            for z in runner["zero_outs"]
        ]
        for a in dev_zeros:
            a.block_until_ready()
        t0 = _time.perf_counter()
        out_arrs = runner["sharded"](*dev_in, *dev_zeros)
        for o in out_arrs:
            o.block_until_ready()
        times.append(_time.perf_counter() - t0)
    results = []
    np_outs = [np.asarray(o) for o in out_arrs]
    for c in range(NCORES):
        results.append(
            {
                name: np_outs[i].reshape(
                    NCORES, *runner["out_avals"][i].shape
                )[c]
                for i, name in enumerate(runner["out_names"])
            }
        )
    return results, times


def kernel(query, key_value, wq, wk, wv, gq, gk, wo, bo, seqlen_q, seqlen_kv):
    global LAST_RESULT

    query = np.asarray(query, np.float32)
    key_value = np.asarray(key_value, np.float32)
    wq = np.asarray(wq, np.float32)
    wk = np.asarray(wk, np.float32)
    wv = np.asarray(wv, np.float32)
    wo = np.asarray(wo, np.float32)
    gq = np.asarray(gq, np.float32)
    gk = np.asarray(gk, np.float32)
    bo = np.asarray(bo, np.float32)
    lq = np.asarray(seqlen_q).astype(np.int64)
    lkv = np.asarray(seqlen_kv).astype(np.int64)

    key = (tuple(lq.tolist()), tuple(lkv.tolist()), MODE)
    if key not in _BUILD_CACHE:
        _BUILD_CACHE[key] = _build(lq, lkv, MODE)
    nc = _BUILD_CACHE[key]

    xqT = np.ascontiguousarray(query.T).astype(BF16)
    xkvT = np.ascontiguousarray(key_value.T).astype(BF16)

    e2ones = np.zeros((128, HPC), BF16)
    for h in range(HPC):
        e2ones[64 * h:64 * (h + 1), h] = 1
    e2gq = np.zeros((HPC, 128), np.float32)
    e2gk = np.zeros((HPC, 128), np.float32)
    for h in range(HPC):
        e2gq[h, 64 * h:64 * (h + 1)] = gq
        e2gk[h, 64 * h:64 * (h + 1)] = gk
    e2gq = e2gq.astype(BF16)
    e2gk = e2gk.astype(BF16)

    in_maps = []
    for c in range(NCORES):
        sl = slice(DPC * c, DPC * (c + 1))
        m = {
            "xqT": xqT,
            "xkvT": xkvT,
            "wqT": np.ascontiguousarray(wq[sl].T).astype(BF16),
            "wkT": np.ascontiguousarray(wk[sl].T).astype(BF16),
            "wvT": np.ascontiguousarray(wv[sl].T).astype(BF16),
            "e2ones": e2ones,
            "e2gq": e2gq,
            "e2gk": e2gk,
        }
        if MODE == "a2a":
            m["woT"] = np.ascontiguousarray(wo.T).astype(BF16)
            m["bo"] = bo
        else:
            m["woTc"] = np.ascontiguousarray(wo[:, sl].T).astype(BF16)
        in_maps.append(m)

    runner = _get_runner(key, nc)
    n_iters = int(os.environ.get("ATTN_KERNEL_ITERS", "1"))
    results, times = _run(runner, in_maps, n_iters=n_iters)
    LAST_RESULT = {"times": times}
    if MODE == "a2a":
        out = np.concatenate([r["out"] for r in results], axis=0)
    else:
        out = results[0]["out"].astype(np.float64)
        for r in results[1:]:
            out = out + r["out"].astype(np.float64)
        out = (out + bo.astype(np.float64)).astype(np.float32)
    return np.asarray(out, np.float32)

